# revision 13
# baseline (speedup 1.0000x reference)
"""Trainium2 Bass kernel for CausalMessagePassingLayer (2x GCN + gated scatter).

Sharding: 8 cores = 4 samples x 2 halves of the embedding dim (D=768 -> 384).
Each core is fully independent (no collectives):
  - out := t  (bf16 t rows cast to f32, written once, fully overlapped)
  - gathers x^T = t[t2x]^T via transposing dma_gather (bf16)
  - xw = x @ W[:, half] on PE (bf16 in, f32 PSUM)
  - y = dinv * xw -> fp8e4, kept in SBUF
  - GCN aggregation z[dst] += #edges * y[src] as dense per-dst-tile adjacency
    matmuls in fp8 DoubleRow perf mode (adjacency counts are small integers,
    exact in fp8e4; self-loops appended on host so z includes dinv*y)
  - e = z * (dinv * tanh(gate)) + tanh(gate)*bias -> f32 rows
  - rows are streamed into the output with dma_scatter_add:
    out[x2t[node]] += e[node]   (tokens not mapped keep out = t)

Host-side work is restricted to index/descriptor preparation (dense adjacency
block counts, degree counts, index wrapping for the DGE) and dtype/layout
marshalling of inputs; all floating-point math runs on device.
"""

import numpy as np
import ml_dtypes

B, S, D, N, E = 4, 8192, 768, 4096, 32768
H = D // 2            # per-core half of embedding dim
P = 128
NT = N // P           # 32 dst tiles per graph
TOKC = S // P         # 64 token chunks
XTG = 512             # tokens per transposed x-gather
OW = 8                # token chunks per out-init wave
SCG = 4               # dst tiles per scatter-add group (512 rows)

bf16 = ml_dtypes.bfloat16
f8e4 = ml_dtypes.float8_e4m3

# test-harness knobs (the grading harness just calls kernel() and these stay default)
TRACE = False
TRACE_CORES = None
LAST_RESULT = None


def _wrap_idx(idx):
    """DGE index layout: i -> [i % 16, i // 16], replicated to 128 partitions."""
    n = idx.shape[0]
    assert n % 16 == 0
    w = idx.astype(np.int16).reshape(n // 16, 16).T
    return np.ascontiguousarray(np.tile(w, (8, 1)))


def _prep_graph(ei):
    """Dense adjacency-count blocks (incl. self loops) + degree counts.

    Returns (A_blocks, deg): A_blocks[t, p, sc, q] = #edges src=sc*128+p ->
    dst=t*128+q, laid out so A_blocks[t] is directly the stack of matmul
    lhsT tiles for dst-tile t. deg includes the self loop. Counts are small
    integers so fp8e4 (e4m3) stores them exactly.
    """
    s = np.concatenate([ei[0].astype(np.int64), np.arange(N, dtype=np.int64)])
    d = np.concatenate([ei[1].astype(np.int64), np.arange(N, dtype=np.int64)])
    A = np.zeros((N, N), np.float32)
    np.add.at(A, (d, s), 1.0)
    deg = np.bincount(d, minlength=N).astype(np.int32)
    Ab = np.ascontiguousarray(
        A.reshape(NT, P, NT, P).transpose(0, 3, 2, 1)
    ).astype(f8e4)
    return Ab, deg


def kernel(**inputs):
    import concourse.bacc as bacc
    import concourse.mybir as mybir
    import concourse.tile as tile
    from concourse.bass_utils import run_bass_kernel_spmd

    f32, bft, fp8, i16, i32 = (
        mybir.dt.float32,
        mybir.dt.bfloat16,
        mybir.dt.float8e4,
        mybir.dt.int16,
        mybir.dt.int32,
    )
    DR = mybir.MatmulPerfMode.DoubleRow

    t_full = np.asarray(inputs["token_embeddings"], dtype=np.float32)
    W = {
        "e": np.asarray(inputs["W_edges"], dtype=np.float32),
        "n": np.asarray(inputs["W_nodes"], dtype=np.float32),
    }
    bias = {
        "e": np.asarray(inputs["b_edges"], dtype=np.float32),
        "n": np.asarray(inputs["b_nodes"], dtype=np.float32),
    }
    gate = {
        "e": np.asarray(inputs["gate_a"], dtype=np.float32).reshape(1, 1),
        "n": np.asarray(inputs["gate_b"], dtype=np.float32).reshape(1, 1),
    }
    t2x = {
        "e": np.asarray(inputs["tokens2edges"], dtype=np.int64),
        "n": np.asarray(inputs["tokens2nodes"], dtype=np.int64),
    }
    x2t = {
        "e": np.asarray(inputs["edges2tokens"], dtype=np.int64),
        "n": np.asarray(inputs["nodes2tokens"], dtype=np.int64),
    }
    ei = {
        "e": np.asarray(inputs["edge_index_edges"], dtype=np.int64),
        "n": np.asarray(inputs["edge_index_nodes"], dtype=np.int64),
    }

    gcns = ("e", "n")

    graphs = {g: [_prep_graph(ei[g][b]) for b in range(B)] for g in gcns}

    # ---- per-core host data ----
    core_data = []
    for b in range(B):
        per_g = {}
        for g in gcns:
            Ab, deg = graphs[g][b]
            # out rows live at r = (tok % 128) * TOKC + tok // 128 so the
            # out-init writes are contiguous per partition; remap the scatter
            # targets accordingly.
            x2t_r = (x2t[g][b] % P) * TOKC + x2t[g][b] // P
            per_g[g] = dict(
                A=Ab,
                deg_pc=np.ascontiguousarray(deg.reshape(NT, P).T),
                t2x_w=_wrap_idx(t2x[g][b]),
                x2t_w=_wrap_idx(x2t_r),
            )
        t_bf = t_full[b].astype(bf16)
        for h in range(2):
            d = dict(t_bf=t_bf)
            for g in gcns:
                d[f"W_{g}"] = np.ascontiguousarray(
                    W[g][:, h * H : (h + 1) * H].reshape(6, P, H).transpose(1, 0, 2)
                ).astype(bf16)
                d[f"bias_{g}"] = np.ascontiguousarray(bias[g][None, h * H : (h + 1) * H])
                d[f"gate_{g}"] = gate[g]
                d.update({f"{k}_{g}": v for k, v in per_g[g].items()})
            d["t_half"] = np.ascontiguousarray(
                t_full[b].reshape(TOKC, P, D).transpose(1, 0, 2)[:, :, h * H : (h + 1) * H]
            ).astype(bf16)
            core_data.append(d)

    # ---- build the SPMD program ----
    nc = bacc.Bacc("TRN2", target_bir_lowering=False, debug=False, num_swdge_queues=1)

    t_bf_d = nc.declare_dram_parameter("t_bf", [S, D], bft, isOutput=False)
    t_half_d = nc.declare_dram_parameter("t_half", [P, TOKC, H], bft, isOutput=False)
    ins_d = {}
    for g in gcns:
        ins_d[f"W_{g}"] = nc.declare_dram_parameter(f"W_{g}", [P, 6, H], bft, isOutput=False)
        ins_d[f"bias_{g}"] = nc.declare_dram_parameter(f"bias_{g}", [1, H], f32, isOutput=False)
        ins_d[f"gate_{g}"] = nc.declare_dram_parameter(f"gate_{g}", [1, 1], f32, isOutput=False)
        ins_d[f"A_{g}"] = nc.declare_dram_parameter(
            f"A_{g}", [NT, P, NT, P], fp8, isOutput=False
        )
        ins_d[f"deg_pc_{g}"] = nc.declare_dram_parameter(
            f"deg_pc_{g}", [P, NT], i32, isOutput=False
        )
        ins_d[f"t2x_w_{g}"] = nc.declare_dram_parameter(
            f"t2x_w_{g}", [P, N // 16], i16, isOutput=False
        )
        ins_d[f"x2t_w_{g}"] = nc.declare_dram_parameter(
            f"x2t_w_{g}", [P, N // 16], i16, isOutput=False
        )
    out_d = nc.declare_dram_parameter("out", [S, H], f32, isOutput=True)

    out_rows = out_d.rearrange("(p c) h -> p c h", p=P)

    with tile.TileContext(nc) as tc:
        with (
            tc.tile_pool(name="cst", bufs=1) as cst,
            tc.tile_pool(name="idxp", bufs=1) as idxp,
            tc.tile_pool(name="xt", bufs=3) as xtp,
            tc.tile_pool(name="xtn", bufs=8) as xtnp,
            tc.tile_pool(name="yp", bufs=2) as yp,
            tc.tile_pool(name="ap", bufs=5) as apool,
            tc.tile_pool(name="ep", bufs=2) as ep,
            tc.tile_pool(name="tb", bufs=2) as tbp,
            tc.tile_pool(name="tf", bufs=1) as tfp,
            tc.tile_pool(name="psxw", bufs=3, space="PSUM") as psxw,
            tc.tile_pool(name="psz", bufs=2, space="PSUM") as psz,
        ):
            # ---------- setup (index tiles first so gathers can start early) ----------
            Wsb, bias_ga, dinv, dinv_ga = {}, {}, {}, {}
            idx_t2x, idx_x2t = {}, {}
            for g in gcns:
                idx_t2x[g] = idxp.tile([P, N // 16], i16, name=f"it2x_{g}", tag=f"it2x_{g}")
                nc.sync.dma_start(out=idx_t2x[g][:], in_=ins_d[f"t2x_w_{g}"][:])
                idx_x2t[g] = idxp.tile([P, N // 16], i16, name=f"ix2t_{g}", tag=f"ix2t_{g}")
                nc.sync.dma_start(out=idx_x2t[g][:], in_=ins_d[f"x2t_w_{g}"][:])
            for g in gcns:
                Wsb[g] = cst.tile([P, 6, H], bft, name=f"W_{g}", tag=f"W_{g}")
                nc.sync.dma_start(out=Wsb[g][:], in_=ins_d[f"W_{g}"][:])

                gcol = cst.tile([P, 1], f32, name=f"gcol_{g}", tag=f"gcol_{g}")
                nc.sync.dma_start(
                    out=gcol[:], in_=ins_d[f"gate_{g}"][:1, :].to_broadcast([P, 1])
                )
                tanh_g = cst.tile([P, 1], f32, name=f"tanh_{g}", tag=f"tanh_{g}")
                nc.scalar.activation(
                    out=tanh_g[:], in_=gcol[:], func=mybir.ActivationFunctionType.Tanh
                )

                brow = cst.tile([P, H], f32, name=f"brow_{g}", tag=f"brow_{g}")
                nc.sync.dma_start(
                    out=brow[:], in_=ins_d[f"bias_{g}"][:1, :].to_broadcast([P, H])
                )
                bias_ga[g] = cst.tile([P, H], f32, name=f"biasga_{g}", tag=f"biasga_{g}")
                nc.vector.tensor_scalar_mul(bias_ga[g][:], brow[:], tanh_g[:, :1])

                deg_i = cst.tile([P, NT], i32, name=f"degi_{g}", tag=f"degi_{g}")
                nc.sync.dma_start(out=deg_i[:], in_=ins_d[f"deg_pc_{g}"][:])
                deg_f = cst.tile([P, NT], f32, name=f"degf_{g}", tag=f"degf_{g}")
                nc.vector.tensor_copy(out=deg_f[:], in_=deg_i[:])
                rdeg = cst.tile([P, NT], f32, name=f"rdeg_{g}", tag=f"rdeg_{g}")
                nc.vector.reciprocal(rdeg[:], deg_f[:])
                dinv[g] = cst.tile([P, NT], f32, name=f"dinv_{g}", tag=f"dinv_{g}")
                nc.scalar.sqrt(dinv[g][:], rdeg[:])
                dinv_ga[g] = cst.tile([P, NT], f32, name=f"dinvga_{g}", tag=f"dinvga_{g}")
                nc.vector.tensor_scalar_mul(dinv_ga[g][:], dinv[g][:], tanh_g[:, :1])

            y_sb = {}
            for g in gcns:
                y_sb[g] = yp.tile([P, NT, H], fp8, name=f"ysb_{g}", tag="ysb")

            def out_init_wave(w):
                """out := t for one wave of OW token chunks (contiguous rows)."""
                tch = tbp.tile([P, OW, H], bft, name="tch", tag="tb")
                nc.sync.dma_start(
                    out=tch[:], in_=t_half_d[:, w * OW : (w + 1) * OW, :]
                )
                tcf = tfp.tile([P, OW, H], f32, name="tcf", tag="tf")
                nc.scalar.activation(
                    out=tcf[:], in_=tch[:], func=mybir.ActivationFunctionType.Copy
                )
                nc.sync.dma_start(
                    out=out_rows[:, w * OW : (w + 1) * OW, :], in_=tcf[:]
                )

            def gather_wave(g, w, pool):
                xT = pool.tile([P, 6, XTG], bft, name="xT")
                nc.gpsimd.dma_gather(
                    out_ap=xT[:],
                    in_ap=t_bf_d[:],
                    idxs_ap=idx_t2x[g][:, w * (XTG // 16) : (w + 1) * (XTG // 16)],
                    num_idxs=XTG,
                    num_idxs_reg=XTG,
                    elem_size=D,
                    transpose=True,
                    queue_num=0,
                )
                return xT

            def xw_chunks(g, w, xT):
                for c in range(XTG // P):
                    cc = w * (XTG // P) + c
                    ps = psxw.tile([P, H], f32)
                    for k in range(6):
                        nc.tensor.matmul(
                            out=ps[:],
                            lhsT=xT[:, k, c * P : (c + 1) * P],
                            rhs=Wsb[g][:, k, :],
                            start=(k == 0),
                            stop=(k == 5),
                        )
                    nc.scalar.activation(
                        out=y_sb[g][:, cc, :],
                        in_=ps[:],
                        func=mybir.ActivationFunctionType.Copy,
                        scale=dinv[g][:, cc : cc + 1],
                    )

            # ---------- aggregation for one dst tile (fp8 DoubleRow) ----------
            def agg_tile(g, t_i, e_grp, j):
                At = apool.tile([P, NT, P], fp8, name="At", tag="At")
                nc.sync.dma_start(out=At[:], in_=ins_d[f"A_{g}"][t_i])
                zt = psz.tile([P, H], f32, name="zt", tag="zt")
                for i in range(NT // 2):
                    nc.tensor.matmul(
                        out=zt[:],
                        lhsT=At[:, 2 * i : 2 * i + 2, :],
                        rhs=y_sb[g][:, 2 * i : 2 * i + 2, :],
                        start=(i == 0),
                        stop=(i == NT // 2 - 1),
                        perf_mode=DR,
                    )
                nc.vector.scalar_tensor_tensor(
                    out=e_grp[:, j, :],
                    in0=zt[:],
                    scalar=dinv_ga[g][:, t_i : t_i + 1],
                    in1=bias_ga[g][:],
                    op0=mybir.AluOpType.mult,
                    op1=mybir.AluOpType.add,
                )

            def agg_grp(g, t0, ntiles):
                """Aggregate dst tiles [t0, t0+ntiles) and scatter-add into out."""
                e_grp = ep.tile([P, ntiles, H], f32, name="e_grp", tag=f"egrp{ntiles}")
                for j in range(ntiles):
                    agg_tile(g, t0 + j, e_grp, j)
                nc.gpsimd.dma_scatter_add(
                    out_ap=out_d[:, :],
                    in_ap=e_grp[:],
                    idxs_ap=idx_x2t[g][
                        :, t0 * (P // 16) : (t0 + ntiles) * (P // 16)
                    ],
                    num_idxs=ntiles * P,
                    num_idxs_reg=ntiles * P,
                    elem_size=H,
                    queue_num=0,
                )

            # phase A: xw for GCN e, out-init waves interleaved
            for w in range(N // XTG):
                xT = gather_wave("e", w, xtp)
                out_init_wave(w)
                xw_chunks("e", w, xT)
            # phase B: prefetch ALL GCN-n gathers before any scatter is enqueued
            # on the gpsimd engine (avoids head-of-line blocking behind scatters)
            xTn = [gather_wave("n", w, xtnp) for w in range(N // XTG)]
            # phase C: interleave xw-n with agg-e groups
            for w in range(N // XTG):
                xw_chunks("n", w, xTn[w])
                agg_grp("e", w * SCG, SCG)
            # phase D: agg-n; split the last group to shrink the end-of-kernel tail
            for grp in range(NT // SCG - 1):
                agg_grp("n", grp * SCG, SCG)
            agg_grp("n", NT - SCG, 2)
            agg_grp("n", NT - 2, 2)

    nc.compile()

    in_maps = [{k: v for k, v in cd.items()} for cd in core_data]
    global LAST_RESULT
    kw = {}
    if TRACE:
        kw = dict(trace=True, trace_cores=TRACE_CORES, stitch_traces=False)
    res = run_bass_kernel_spmd(nc, in_maps, list(range(8)), **kw)
    LAST_RESULT = res

    out = np.empty((B, S, D), np.float32)
    for b in range(B):
        for h in range(2):
            o = np.asarray(res.results[2 * b + h]["out"], dtype=np.float32)
            out[b, :, h * H : (h + 1) * H] = (
                o.reshape(P, TOKC, H).transpose(1, 0, 2).reshape(S, H)
            )
    return out


# revision 14
# speedup vs baseline: 1.0196x; 1.0196x over previous
"""Trainium2 Bass kernel for CausalMessagePassingLayer (2x GCN + gated scatter).

Sharding: 8 cores = 4 samples x 2 halves of the embedding dim (D=768 -> 384).
Each core is fully independent (no collectives):
  - out := t  (bf16 t rows cast to f32, written once, fully overlapped)
  - gathers x^T = t[t2x]^T via transposing dma_gather (bf16)
  - xw = x @ W[:, half] on PE (bf16 in, f32 PSUM)
  - y = dinv * xw -> fp8e4, kept in SBUF
  - GCN aggregation z[dst] += #edges * y[src] as dense per-dst-tile adjacency
    matmuls in fp8 DoubleRow perf mode (adjacency counts are small integers,
    exact in fp8e4; self-loops appended on host so z includes dinv*y)
  - e = z * (dinv * tanh(gate)) + tanh(gate)*bias -> f32 rows
  - rows are streamed into the output with dma_scatter_add:
    out[x2t[node]] += e[node]   (tokens not mapped keep out = t)

Host-side work is restricted to index/descriptor preparation (dense adjacency
block counts, degree counts, index wrapping for the DGE) and dtype/layout
marshalling of inputs; all floating-point math runs on device.
"""

import numpy as np
import ml_dtypes

B, S, D, N, E = 4, 8192, 768, 4096, 32768
H = D // 2            # per-core half of embedding dim
P = 128
NT = N // P           # 32 dst tiles per graph
TOKC = S // P         # 64 token chunks
XTG = 512             # tokens per transposed x-gather
OW = 8                # token chunks per out-init wave
SCG = 4               # dst tiles per scatter-add group (512 rows)

bf16 = ml_dtypes.bfloat16
f8e4 = ml_dtypes.float8_e4m3

# test-harness knobs (the grading harness just calls kernel() and these stay default)
TRACE = False
TRACE_CORES = None
LAST_RESULT = None


def _wrap_idx(idx):
    """DGE index layout: i -> [i % 16, i // 16], replicated to 128 partitions."""
    n = idx.shape[0]
    assert n % 16 == 0
    w = idx.astype(np.int16).reshape(n // 16, 16).T
    return np.ascontiguousarray(np.tile(w, (8, 1)))


def _prep_graph(ei):
    """Dense adjacency-count blocks (incl. self loops) + degree counts.

    Returns (A_blocks, deg): A_blocks[t, p, sc, q] = #edges src=sc*128+p ->
    dst=t*128+q, laid out so A_blocks[t] is directly the stack of matmul
    lhsT tiles for dst-tile t. deg includes the self loop. Counts are small
    integers so fp8e4 (e4m3) stores them exactly.
    """
    s = np.concatenate([ei[0].astype(np.int64), np.arange(N, dtype=np.int64)])
    d = np.concatenate([ei[1].astype(np.int64), np.arange(N, dtype=np.int64)])
    A = np.zeros((N, N), np.float32)
    np.add.at(A, (d, s), 1.0)
    deg = np.bincount(d, minlength=N).astype(np.int32)
    Ab = np.ascontiguousarray(
        A.reshape(NT, P, NT, P).transpose(0, 3, 2, 1)
    ).astype(f8e4)
    return Ab, deg


def kernel(**inputs):
    import concourse.bacc as bacc
    import concourse.mybir as mybir
    import concourse.tile as tile
    from concourse.bass_utils import run_bass_kernel_spmd

    f32, bft, fp8, i16, i32 = (
        mybir.dt.float32,
        mybir.dt.bfloat16,
        mybir.dt.float8e4,
        mybir.dt.int16,
        mybir.dt.int32,
    )
    DR = mybir.MatmulPerfMode.DoubleRow

    t_full = np.asarray(inputs["token_embeddings"], dtype=np.float32)
    W = {
        "e": np.asarray(inputs["W_edges"], dtype=np.float32),
        "n": np.asarray(inputs["W_nodes"], dtype=np.float32),
    }
    bias = {
        "e": np.asarray(inputs["b_edges"], dtype=np.float32),
        "n": np.asarray(inputs["b_nodes"], dtype=np.float32),
    }
    gate = {
        "e": np.asarray(inputs["gate_a"], dtype=np.float32).reshape(1, 1),
        "n": np.asarray(inputs["gate_b"], dtype=np.float32).reshape(1, 1),
    }
    t2x = {
        "e": np.asarray(inputs["tokens2edges"], dtype=np.int64),
        "n": np.asarray(inputs["tokens2nodes"], dtype=np.int64),
    }
    x2t = {
        "e": np.asarray(inputs["edges2tokens"], dtype=np.int64),
        "n": np.asarray(inputs["nodes2tokens"], dtype=np.int64),
    }
    ei = {
        "e": np.asarray(inputs["edge_index_edges"], dtype=np.int64),
        "n": np.asarray(inputs["edge_index_nodes"], dtype=np.int64),
    }

    gcns = ("e", "n")

    graphs = {g: [_prep_graph(ei[g][b]) for b in range(B)] for g in gcns}

    # ---- per-core host data ----
    core_data = []
    for b in range(B):
        per_g = {}
        for g in gcns:
            Ab, deg = graphs[g][b]
            # out rows live at r = (tok % 128) * TOKC + tok // 128 so the
            # out-init writes are contiguous per partition; remap the scatter
            # targets accordingly.
            x2t_r = (x2t[g][b] % P) * TOKC + x2t[g][b] // P
            per_g[g] = dict(
                A=Ab,
                deg_pc=np.ascontiguousarray(deg.reshape(NT, P).T),
                t2x_w=_wrap_idx(t2x[g][b]),
                x2t_w=_wrap_idx(x2t_r),
            )
        t_bf = t_full[b].astype(bf16)
        for h in range(2):
            d = dict(t_bf=t_bf)
            for g in gcns:
                d[f"W_{g}"] = np.ascontiguousarray(
                    W[g][:, h * H : (h + 1) * H].reshape(6, P, H).transpose(1, 0, 2)
                ).astype(bf16)
                d[f"bias_{g}"] = np.ascontiguousarray(bias[g][None, h * H : (h + 1) * H])
                d[f"gate_{g}"] = gate[g]
                d.update({f"{k}_{g}": v for k, v in per_g[g].items()})
            d["t_half"] = np.ascontiguousarray(
                t_full[b].reshape(TOKC, P, D).transpose(1, 0, 2)[:, :, h * H : (h + 1) * H]
            ).astype(bf16)
            core_data.append(d)

    # ---- build the SPMD program ----
    nc = bacc.Bacc("TRN2", target_bir_lowering=False, debug=False, num_swdge_queues=1)

    t_bf_d = nc.declare_dram_parameter("t_bf", [S, D], bft, isOutput=False)
    t_half_d = nc.declare_dram_parameter("t_half", [P, TOKC, H], bft, isOutput=False)
    ins_d = {}
    for g in gcns:
        ins_d[f"W_{g}"] = nc.declare_dram_parameter(f"W_{g}", [P, 6, H], bft, isOutput=False)
        ins_d[f"bias_{g}"] = nc.declare_dram_parameter(f"bias_{g}", [1, H], f32, isOutput=False)
        ins_d[f"gate_{g}"] = nc.declare_dram_parameter(f"gate_{g}", [1, 1], f32, isOutput=False)
        ins_d[f"A_{g}"] = nc.declare_dram_parameter(
            f"A_{g}", [NT, P, NT, P], fp8, isOutput=False
        )
        ins_d[f"deg_pc_{g}"] = nc.declare_dram_parameter(
            f"deg_pc_{g}", [P, NT], i32, isOutput=False
        )
        ins_d[f"t2x_w_{g}"] = nc.declare_dram_parameter(
            f"t2x_w_{g}", [P, N // 16], i16, isOutput=False
        )
        ins_d[f"x2t_w_{g}"] = nc.declare_dram_parameter(
            f"x2t_w_{g}", [P, N // 16], i16, isOutput=False
        )
    out_d = nc.declare_dram_parameter("out", [S, H], f32, isOutput=True)

    out_rows = out_d.rearrange("(p c) h -> p c h", p=P)

    with tile.TileContext(nc) as tc:
        with (
            tc.tile_pool(name="cst", bufs=1) as cst,
            tc.tile_pool(name="idxp", bufs=1) as idxp,
            tc.tile_pool(name="xt", bufs=3) as xtp,
            tc.tile_pool(name="xtn", bufs=8) as xtnp,
            tc.tile_pool(name="yp", bufs=2) as yp,
            tc.tile_pool(name="ap", bufs=5) as apool,
            tc.tile_pool(name="ep", bufs=2) as ep,
            tc.tile_pool(name="tb", bufs=2) as tbp,
            tc.tile_pool(name="tf", bufs=1) as tfp,
            tc.tile_pool(name="psxw", bufs=3, space="PSUM") as psxw,
            tc.tile_pool(name="psz", bufs=2, space="PSUM") as psz,
        ):
            # ---------- setup (index tiles first so gathers can start early) ----------
            Wsb, bias_ga, dinv, dinv_ga = {}, {}, {}, {}
            idx_t2x, idx_x2t = {}, {}
            for g in gcns:
                idx_t2x[g] = idxp.tile([P, N // 16], i16, name=f"it2x_{g}", tag=f"it2x_{g}")
                nc.sync.dma_start(out=idx_t2x[g][:], in_=ins_d[f"t2x_w_{g}"][:])
                idx_x2t[g] = idxp.tile([P, N // 16], i16, name=f"ix2t_{g}", tag=f"ix2t_{g}")
                nc.sync.dma_start(out=idx_x2t[g][:], in_=ins_d[f"x2t_w_{g}"][:])
            for g in gcns:
                Wsb[g] = cst.tile([P, 6, H], bft, name=f"W_{g}", tag=f"W_{g}")
                nc.sync.dma_start(out=Wsb[g][:], in_=ins_d[f"W_{g}"][:])

                gcol = cst.tile([P, 1], f32, name=f"gcol_{g}", tag=f"gcol_{g}")
                nc.sync.dma_start(
                    out=gcol[:], in_=ins_d[f"gate_{g}"][:1, :].to_broadcast([P, 1])
                )
                tanh_g = cst.tile([P, 1], f32, name=f"tanh_{g}", tag=f"tanh_{g}")
                nc.scalar.activation(
                    out=tanh_g[:], in_=gcol[:], func=mybir.ActivationFunctionType.Tanh
                )

                brow = cst.tile([P, H], f32, name=f"brow_{g}", tag=f"brow_{g}")
                nc.sync.dma_start(
                    out=brow[:], in_=ins_d[f"bias_{g}"][:1, :].to_broadcast([P, H])
                )
                bias_ga[g] = cst.tile([P, H], f32, name=f"biasga_{g}", tag=f"biasga_{g}")
                nc.vector.tensor_scalar_mul(bias_ga[g][:], brow[:], tanh_g[:, :1])

                deg_i = cst.tile([P, NT], i32, name=f"degi_{g}", tag=f"degi_{g}")
                nc.sync.dma_start(out=deg_i[:], in_=ins_d[f"deg_pc_{g}"][:])
                deg_f = cst.tile([P, NT], f32, name=f"degf_{g}", tag=f"degf_{g}")
                nc.vector.tensor_copy(out=deg_f[:], in_=deg_i[:])
                rdeg = cst.tile([P, NT], f32, name=f"rdeg_{g}", tag=f"rdeg_{g}")
                nc.vector.reciprocal(rdeg[:], deg_f[:])
                dinv[g] = cst.tile([P, NT], f32, name=f"dinv_{g}", tag=f"dinv_{g}")
                nc.scalar.sqrt(dinv[g][:], rdeg[:])
                dinv_ga[g] = cst.tile([P, NT], f32, name=f"dinvga_{g}", tag=f"dinvga_{g}")
                nc.vector.tensor_scalar_mul(dinv_ga[g][:], dinv[g][:], tanh_g[:, :1])

            y_sb = {}
            for g in gcns:
                y_sb[g] = yp.tile([P, NT, H], fp8, name=f"ysb_{g}", tag="ysb")

            def out_init_wave(w):
                """out := t for one wave of OW token chunks (contiguous rows)."""
                tch = tbp.tile([P, OW, H], bft, name="tch", tag="tb")
                nc.sync.dma_start(
                    out=tch[:], in_=t_half_d[:, w * OW : (w + 1) * OW, :]
                )
                tcf = tfp.tile([P, OW, H], f32, name="tcf", tag="tf")
                nc.scalar.activation(
                    out=tcf[:], in_=tch[:], func=mybir.ActivationFunctionType.Copy
                )
                nc.sync.dma_start(
                    out=out_rows[:, w * OW : (w + 1) * OW, :], in_=tcf[:]
                )

            def gather_wave(g, w, pool):
                xT = pool.tile([P, 6, XTG], bft, name="xT")
                nc.gpsimd.dma_gather(
                    out_ap=xT[:],
                    in_ap=t_bf_d[:],
                    idxs_ap=idx_t2x[g][:, w * (XTG // 16) : (w + 1) * (XTG // 16)],
                    num_idxs=XTG,
                    num_idxs_reg=XTG,
                    elem_size=D,
                    transpose=True,
                    queue_num=0,
                )
                return xT

            def xw_chunks(g, w, xT):
                for c in range(XTG // P):
                    cc = w * (XTG // P) + c
                    ps = psxw.tile([P, H], f32)
                    for k in range(6):
                        nc.tensor.matmul(
                            out=ps[:],
                            lhsT=xT[:, k, c * P : (c + 1) * P],
                            rhs=Wsb[g][:, k, :],
                            start=(k == 0),
                            stop=(k == 5),
                        )
                    nc.scalar.activation(
                        out=y_sb[g][:, cc, :],
                        in_=ps[:],
                        func=mybir.ActivationFunctionType.Copy,
                        scale=dinv[g][:, cc : cc + 1],
                    )

            # ---------- aggregation for one dst tile (fp8 DoubleRow) ----------
            def agg_tile(g, t_i, e_grp, j):
                At = apool.tile([P, NT, P], fp8, name="At", tag="At")
                nc.sync.dma_start(out=At[:], in_=ins_d[f"A_{g}"][t_i])
                zt = psz.tile([P, H], f32, name="zt", tag="zt")
                for i in range(NT // 2):
                    nc.tensor.matmul(
                        out=zt[:],
                        lhsT=At[:, 2 * i : 2 * i + 2, :],
                        rhs=y_sb[g][:, 2 * i : 2 * i + 2, :],
                        start=(i == 0),
                        stop=(i == NT // 2 - 1),
                        perf_mode=DR,
                    )
                nc.vector.scalar_tensor_tensor(
                    out=e_grp[:, j, :],
                    in0=zt[:],
                    scalar=dinv_ga[g][:, t_i : t_i + 1],
                    in1=bias_ga[g][:],
                    op0=mybir.AluOpType.mult,
                    op1=mybir.AluOpType.add,
                )

            def agg_grp(g, t0, ntiles):
                """Aggregate dst tiles [t0, t0+ntiles) and scatter-add into out."""
                e_grp = ep.tile([P, ntiles, H], f32, name="e_grp", tag=f"egrp{ntiles}")
                for j in range(ntiles):
                    agg_tile(g, t0 + j, e_grp, j)
                nc.gpsimd.dma_scatter_add(
                    out_ap=out_d[:, :],
                    in_ap=e_grp[:],
                    idxs_ap=idx_x2t[g][
                        :, t0 * (P // 16) : (t0 + ntiles) * (P // 16)
                    ],
                    num_idxs=ntiles * P,
                    num_idxs_reg=ntiles * P,
                    elem_size=H,
                    queue_num=0,
                )

            # phase 0: out := t up front — every scatter-add WAW-depends on the
            # last of these writes, so they must complete early
            for w in range(TOKC // OW):
                out_init_wave(w)
            # phase A: xw for GCN e
            for w in range(N // XTG):
                xT = gather_wave("e", w, xtp)
                xw_chunks("e", w, xT)
            # phase B: prefetch ALL GCN-n gathers before any scatter is enqueued
            # on the gpsimd engine (avoids head-of-line blocking behind scatters)
            xTn = [gather_wave("n", w, xtnp) for w in range(N // XTG)]
            # phase C: interleave xw-n with agg-e groups
            for w in range(N // XTG):
                xw_chunks("n", w, xTn[w])
                agg_grp("e", w * SCG, SCG)
            # phase D: agg-n; split the last group to shrink the end-of-kernel tail
            for grp in range(NT // SCG - 1):
                agg_grp("n", grp * SCG, SCG)
            agg_grp("n", NT - SCG, 2)
            agg_grp("n", NT - 2, 2)

    nc.compile()

    in_maps = [{k: v for k, v in cd.items()} for cd in core_data]
    global LAST_RESULT
    kw = {}
    if TRACE:
        kw = dict(trace=True, trace_cores=TRACE_CORES, stitch_traces=False)
    res = run_bass_kernel_spmd(nc, in_maps, list(range(8)), **kw)
    LAST_RESULT = res

    out = np.empty((B, S, D), np.float32)
    for b in range(B):
        for h in range(2):
            o = np.asarray(res.results[2 * b + h]["out"], dtype=np.float32)
            out[b, :, h * H : (h + 1) * H] = (
                o.reshape(P, TOKC, H).transpose(1, 0, 2).reshape(S, H)
            )
    return out


# revision 19
# speedup vs baseline: 1.2370x; 1.2132x over previous
"""Trainium2 Bass kernel for CausalMessagePassingLayer (2x GCN + gated scatter).

Sharding: 8 cores = 4 samples x 2 halves of the embedding dim (D=768 -> 384).
Each core is fully independent (no collectives):
  - out := t  (bf16 t rows cast to f32, written once, fully overlapped)
  - gathers x^T = t[t2x]^T via transposing dma_gather (bf16)
  - xw = x @ W[:, half] on PE (bf16 in, f32 PSUM)
  - y = dinv * xw -> fp8e4, kept in SBUF
  - GCN aggregation z[dst] += #edges * y[src] as dense per-dst-tile adjacency
    matmuls in fp8 DoubleRow perf mode (adjacency counts are small integers,
    exact in fp8e4; self-loops appended on host so z includes dinv*y)
  - e = z * (dinv * tanh(gate)) + tanh(gate)*bias -> f32 rows
  - rows are streamed into the output with dma_scatter_add:
    out[x2t[node]] += e[node]   (tokens not mapped keep out = t)

Host-side work is restricted to index/descriptor preparation (dense adjacency
block counts, degree counts, index wrapping for the DGE) and dtype/layout
marshalling of inputs; all floating-point math runs on device.
"""

import numpy as np
import ml_dtypes

B, S, D, N, E = 4, 8192, 768, 4096, 32768
H = D // 2            # per-core half of embedding dim
P = 128
NT = N // P           # 32 dst tiles per graph
TOKC = S // P         # 64 token chunks
XTG = 512             # tokens per transposed x-gather
OW = 8                # token chunks per out-init wave
SCG = 4               # dst tiles per scatter-add group (512 rows)

bf16 = ml_dtypes.bfloat16
f8e4 = ml_dtypes.float8_e4m3

# test-harness knobs (the grading harness just calls kernel() and these stay default)
TRACE = False
TRACE_CORES = None
LAST_RESULT = None


def _wrap_idx(idx):
    """DGE index layout: i -> [i % 16, i // 16], replicated to 128 partitions."""
    n = idx.shape[0]
    assert n % 16 == 0
    w = idx.astype(np.int16).reshape(n // 16, 16).T
    return np.ascontiguousarray(np.tile(w, (8, 1)))


def _prep_graph(ei):
    """Dense adjacency-count blocks (incl. self loops) + degree counts.

    Returns (A_blocks, deg): A_blocks[t, p, sc, q] = #edges src=sc*128+p ->
    dst=t*128+q, laid out so A_blocks[t] is directly the stack of matmul
    lhsT tiles for dst-tile t. deg includes the self loop. Counts are small
    integers so fp8e4 (e4m3) stores them exactly.
    """
    s = np.concatenate([ei[0].astype(np.int64), np.arange(N, dtype=np.int64)])
    d = np.concatenate([ei[1].astype(np.int64), np.arange(N, dtype=np.int64)])
    A = np.zeros((N, N), np.float32)
    np.add.at(A, (d, s), 1.0)
    deg = np.bincount(d, minlength=N).astype(np.int32)
    Ab = np.ascontiguousarray(
        A.reshape(NT, P, NT, P).transpose(0, 3, 2, 1)
    ).astype(f8e4)
    return Ab, deg


def kernel(**inputs):
    import concourse.bacc as bacc
    import concourse.mybir as mybir
    import concourse.tile as tile
    from concourse.bass_utils import run_bass_kernel_spmd

    f32, bft, fp8, i16, i32 = (
        mybir.dt.float32,
        mybir.dt.bfloat16,
        mybir.dt.float8e4,
        mybir.dt.int16,
        mybir.dt.int32,
    )
    DR = mybir.MatmulPerfMode.DoubleRow

    t_full = np.asarray(inputs["token_embeddings"], dtype=np.float32)
    W = {
        "e": np.asarray(inputs["W_edges"], dtype=np.float32),
        "n": np.asarray(inputs["W_nodes"], dtype=np.float32),
    }
    bias = {
        "e": np.asarray(inputs["b_edges"], dtype=np.float32),
        "n": np.asarray(inputs["b_nodes"], dtype=np.float32),
    }
    gate = {
        "e": np.asarray(inputs["gate_a"], dtype=np.float32).reshape(1, 1),
        "n": np.asarray(inputs["gate_b"], dtype=np.float32).reshape(1, 1),
    }
    t2x = {
        "e": np.asarray(inputs["tokens2edges"], dtype=np.int64),
        "n": np.asarray(inputs["tokens2nodes"], dtype=np.int64),
    }
    x2t = {
        "e": np.asarray(inputs["edges2tokens"], dtype=np.int64),
        "n": np.asarray(inputs["nodes2tokens"], dtype=np.int64),
    }
    ei = {
        "e": np.asarray(inputs["edge_index_edges"], dtype=np.int64),
        "n": np.asarray(inputs["edge_index_nodes"], dtype=np.int64),
    }

    gcns = ("e", "n")

    graphs = {g: [_prep_graph(ei[g][b]) for b in range(B)] for g in gcns}

    # ---- per-core host data ----
    core_data = []
    for b in range(B):
        per_g = {}
        for g in gcns:
            Ab, deg = graphs[g][b]
            # out rows live at r = (tok % 128) * TOKC + tok // 128 so the
            # out-init writes are contiguous per partition; remap the scatter
            # targets accordingly.
            x2t_r = (x2t[g][b] % P) * TOKC + x2t[g][b] // P
            per_g[g] = dict(
                A=Ab,
                deg_pc=np.ascontiguousarray(deg.reshape(NT, P).T),
                t2x_w=_wrap_idx(t2x[g][b]),
                x2t_w=_wrap_idx(x2t_r),
            )
        t_bf = t_full[b].astype(bf16)
        for h in range(2):
            d = dict(t_bf=t_bf)
            for g in gcns:
                d[f"W_{g}"] = np.ascontiguousarray(
                    W[g][:, h * H : (h + 1) * H].reshape(6, P, H).transpose(1, 0, 2)
                ).astype(bf16)
                d[f"bias_{g}"] = np.ascontiguousarray(bias[g][None, h * H : (h + 1) * H])
                d[f"gate_{g}"] = gate[g]
                d.update({f"{k}_{g}": v for k, v in per_g[g].items()})
            d["t_half"] = np.ascontiguousarray(
                t_full[b].reshape(TOKC, P, D).transpose(1, 0, 2)[:, :, h * H : (h + 1) * H]
            ).astype(bf16)
            core_data.append(d)

    # ---- build the SPMD program ----
    nc = bacc.Bacc("TRN2", target_bir_lowering=False, debug=False, num_swdge_queues=1)

    t_bf_d = nc.declare_dram_parameter("t_bf", [S, D], bft, isOutput=False)
    t_half_d = nc.declare_dram_parameter("t_half", [P, TOKC, H], bft, isOutput=False)
    ins_d = {}
    for g in gcns:
        ins_d[f"W_{g}"] = nc.declare_dram_parameter(f"W_{g}", [P, 6, H], bft, isOutput=False)
        ins_d[f"bias_{g}"] = nc.declare_dram_parameter(f"bias_{g}", [1, H], f32, isOutput=False)
        ins_d[f"gate_{g}"] = nc.declare_dram_parameter(f"gate_{g}", [1, 1], f32, isOutput=False)
        ins_d[f"A_{g}"] = nc.declare_dram_parameter(
            f"A_{g}", [NT, P, NT, P], fp8, isOutput=False
        )
        ins_d[f"deg_pc_{g}"] = nc.declare_dram_parameter(
            f"deg_pc_{g}", [P, NT], i32, isOutput=False
        )
        ins_d[f"t2x_w_{g}"] = nc.declare_dram_parameter(
            f"t2x_w_{g}", [P, N // 16], i16, isOutput=False
        )
        ins_d[f"x2t_w_{g}"] = nc.declare_dram_parameter(
            f"x2t_w_{g}", [P, N // 16], i16, isOutput=False
        )
    out_d = nc.declare_dram_parameter("out", [S, H], bft, isOutput=True)

    out_rows = out_d.rearrange("(p c) h -> p c h", p=P)

    with tile.TileContext(nc) as tc:
        with (
            tc.tile_pool(name="cst", bufs=1) as cst,
            tc.tile_pool(name="idxp", bufs=1) as idxp,
            tc.tile_pool(name="xt", bufs=3) as xtp,
            tc.tile_pool(name="xtn", bufs=8) as xtnp,
            tc.tile_pool(name="yp", bufs=2) as yp,
            tc.tile_pool(name="ap", bufs=5) as apool,
            tc.tile_pool(name="ep", bufs=2) as ep,
            tc.tile_pool(name="tb", bufs=4) as tbp,
            tc.tile_pool(name="psxw", bufs=3, space="PSUM") as psxw,
            tc.tile_pool(name="psz", bufs=2, space="PSUM") as psz,
        ):
            # ---------- setup (index tiles first so gathers can start early) ----------
            Wsb, bias_ga, dinv, dinv_ga = {}, {}, {}, {}
            idx_t2x, idx_x2t = {}, {}
            for g in gcns:
                idx_t2x[g] = idxp.tile([P, N // 16], i16, name=f"it2x_{g}", tag=f"it2x_{g}")
                nc.sync.dma_start(out=idx_t2x[g][:], in_=ins_d[f"t2x_w_{g}"][:])
                idx_x2t[g] = idxp.tile([P, N // 16], i16, name=f"ix2t_{g}", tag=f"ix2t_{g}")
                nc.sync.dma_start(out=idx_x2t[g][:], in_=ins_d[f"x2t_w_{g}"][:])
            for g in gcns:
                Wsb[g] = cst.tile([P, 6, H], bft, name=f"W_{g}", tag=f"W_{g}")
                nc.sync.dma_start(out=Wsb[g][:], in_=ins_d[f"W_{g}"][:])

                gcol = cst.tile([P, 1], f32, name=f"gcol_{g}", tag=f"gcol_{g}")
                nc.sync.dma_start(
                    out=gcol[:], in_=ins_d[f"gate_{g}"][:1, :].to_broadcast([P, 1])
                )
                tanh_g = cst.tile([P, 1], f32, name=f"tanh_{g}", tag=f"tanh_{g}")
                nc.scalar.activation(
                    out=tanh_g[:], in_=gcol[:], func=mybir.ActivationFunctionType.Tanh
                )

                brow = cst.tile([P, H], f32, name=f"brow_{g}", tag=f"brow_{g}")
                nc.sync.dma_start(
                    out=brow[:], in_=ins_d[f"bias_{g}"][:1, :].to_broadcast([P, H])
                )
                bias_ga[g] = cst.tile([P, H], f32, name=f"biasga_{g}", tag=f"biasga_{g}")
                nc.vector.tensor_scalar_mul(bias_ga[g][:], brow[:], tanh_g[:, :1])

                deg_i = cst.tile([P, NT], i32, name=f"degi_{g}", tag=f"degi_{g}")
                nc.sync.dma_start(out=deg_i[:], in_=ins_d[f"deg_pc_{g}"][:])
                deg_f = cst.tile([P, NT], f32, name=f"degf_{g}", tag=f"degf_{g}")
                nc.vector.tensor_copy(out=deg_f[:], in_=deg_i[:])
                rdeg = cst.tile([P, NT], f32, name=f"rdeg_{g}", tag=f"rdeg_{g}")
                nc.vector.reciprocal(rdeg[:], deg_f[:])
                dinv[g] = cst.tile([P, NT], f32, name=f"dinv_{g}", tag=f"dinv_{g}")
                nc.scalar.sqrt(dinv[g][:], rdeg[:])
                dinv_ga[g] = cst.tile([P, NT], f32, name=f"dinvga_{g}", tag=f"dinvga_{g}")
                nc.vector.tensor_scalar_mul(dinv_ga[g][:], dinv[g][:], tanh_g[:, :1])

            y_sb = {}
            for g in gcns:
                y_sb[g] = yp.tile([P, NT, H], fp8, name=f"ysb_{g}", tag="ysb")

            def out_init_wave(w):
                """out := t for one wave of OW token chunks (contiguous rows)."""
                tch = tbp.tile([P, OW, H], bft, name="tch", tag="tb")
                nc.sync.dma_start(
                    out=tch[:], in_=t_half_d[:, w * OW : (w + 1) * OW, :]
                )
                nc.sync.dma_start(
                    out=out_rows[:, w * OW : (w + 1) * OW, :], in_=tch[:]
                )

            def gather_wave(g, w, pool):
                xT = pool.tile([P, 6, XTG], bft, name="xT")
                nc.gpsimd.dma_gather(
                    out_ap=xT[:],
                    in_ap=t_bf_d[:],
                    idxs_ap=idx_t2x[g][:, w * (XTG // 16) : (w + 1) * (XTG // 16)],
                    num_idxs=XTG,
                    num_idxs_reg=XTG,
                    elem_size=D,
                    transpose=True,
                    queue_num=0,
                )
                return xT

            def xw_chunks(g, w, xT):
                for c in range(XTG // P):
                    cc = w * (XTG // P) + c
                    ps = psxw.tile([P, H], f32)
                    for k in range(6):
                        nc.tensor.matmul(
                            out=ps[:],
                            lhsT=xT[:, k, c * P : (c + 1) * P],
                            rhs=Wsb[g][:, k, :],
                            start=(k == 0),
                            stop=(k == 5),
                        )
                    nc.scalar.activation(
                        out=y_sb[g][:, cc, :],
                        in_=ps[:],
                        func=mybir.ActivationFunctionType.Copy,
                        scale=dinv[g][:, cc : cc + 1],
                    )

            # ---------- aggregation for one dst tile (fp8 DoubleRow) ----------
            def agg_tile(g, t_i, e_grp, j):
                At = apool.tile([P, NT, P], fp8, name="At", tag="At")
                nc.sync.dma_start(out=At[:], in_=ins_d[f"A_{g}"][t_i])
                zt = psz.tile([P, H], f32, name="zt", tag="zt")
                for i in range(NT // 2):
                    nc.tensor.matmul(
                        out=zt[:],
                        lhsT=At[:, 2 * i : 2 * i + 2, :],
                        rhs=y_sb[g][:, 2 * i : 2 * i + 2, :],
                        start=(i == 0),
                        stop=(i == NT // 2 - 1),
                        perf_mode=DR,
                    )
                nc.vector.scalar_tensor_tensor(
                    out=e_grp[:, j, :],
                    in0=zt[:],
                    scalar=dinv_ga[g][:, t_i : t_i + 1],
                    in1=bias_ga[g][:],
                    op0=mybir.AluOpType.mult,
                    op1=mybir.AluOpType.add,
                )

            def agg_grp(g, t0, ntiles):
                """Aggregate dst tiles [t0, t0+ntiles) and scatter-add into out."""
                e_grp = ep.tile([P, ntiles, H], bft, name="e_grp", tag=f"egrp{ntiles}")
                for j in range(ntiles):
                    agg_tile(g, t0 + j, e_grp, j)
                nc.gpsimd.dma_scatter_add(
                    out_ap=out_d[:, :],
                    in_ap=e_grp[:],
                    idxs_ap=idx_x2t[g][
                        :, t0 * (P // 16) : (t0 + ntiles) * (P // 16)
                    ],
                    num_idxs=ntiles * P,
                    num_idxs_reg=ntiles * P,
                    elem_size=H,
                    queue_num=0,
                )

            # phase 0: out := t up front — every scatter-add WAW-depends on the
            # last of these writes, so they must complete early
            for w in range(TOKC // OW):
                out_init_wave(w)
            # phase A: xw for GCN e
            for w in range(N // XTG):
                xT = gather_wave("e", w, xtp)
                xw_chunks("e", w, xT)
            # phase B: prefetch ALL GCN-n gathers before any scatter is enqueued
            # on the gpsimd engine (avoids head-of-line blocking behind scatters)
            xTn = [gather_wave("n", w, xtnp) for w in range(N // XTG)]
            # phase C: interleave xw-n with agg-e groups
            for w in range(N // XTG):
                xw_chunks("n", w, xTn[w])
                agg_grp("e", w * SCG, SCG)
            # phase D: agg-n
            for grp in range(NT // SCG):
                agg_grp("n", grp * SCG, SCG)

    nc.compile()

    in_maps = [{k: v for k, v in cd.items()} for cd in core_data]
    global LAST_RESULT
    kw = {}
    if TRACE:
        kw = dict(trace=True, trace_cores=TRACE_CORES, stitch_traces=False)
    res = run_bass_kernel_spmd(nc, in_maps, list(range(8)), **kw)
    LAST_RESULT = res

    out = np.empty((B, S, D), np.float32)
    for b in range(B):
        for h in range(2):
            o = np.asarray(res.results[2 * b + h]["out"], dtype=np.float32)
            out[b, :, h * H : (h + 1) * H] = (
                o.reshape(P, TOKC, H).transpose(1, 0, 2).reshape(S, H)
            )
    return out


# revision 26
# speedup vs baseline: 1.2403x; 1.0026x over previous
"""Trainium2 Bass kernel for CausalMessagePassingLayer (2x GCN + gated scatter).

Sharding: 8 cores = 4 samples x 2 halves of the embedding dim (D=768 -> 384).
Each core is fully independent (no collectives):
  - out := t  (bf16 t rows cast to f32, written once, fully overlapped)
  - gathers x^T = t[t2x]^T via transposing dma_gather (bf16)
  - xw = x @ W[:, half] on PE (bf16 in, f32 PSUM)
  - y = dinv * xw -> fp8e4, kept in SBUF
  - GCN aggregation z[dst] += #edges * y[src] as dense per-dst-tile adjacency
    matmuls in fp8 DoubleRow perf mode (adjacency counts are small integers,
    exact in fp8e4; self-loops appended on host so z includes dinv*y)
  - e = z * (dinv * tanh(gate)) + tanh(gate)*bias -> f32 rows
  - rows are streamed into the output with dma_scatter_add:
    out[x2t[node]] += e[node]   (tokens not mapped keep out = t)

Host-side work is restricted to index/descriptor preparation (dense adjacency
block counts, degree counts, index wrapping for the DGE) and dtype/layout
marshalling of inputs; all floating-point math runs on device.
"""

import numpy as np
import ml_dtypes

B, S, D, N, E = 4, 8192, 768, 4096, 32768
H = D // 2            # per-core half of embedding dim
P = 128
NT = N // P           # 32 dst tiles per graph
TOKC = S // P         # 64 token chunks
XTG = 512             # tokens per transposed x-gather
OW = 8                # token chunks per out-init wave
SCG = 4               # dst tiles per scatter-add group (512 rows)

bf16 = ml_dtypes.bfloat16
f8e4 = ml_dtypes.float8_e4m3

# test-harness knobs (the grading harness just calls kernel() and these stay default)
TRACE = False
TRACE_CORES = None
LAST_RESULT = None


def _wrap_idx(idx):
    """DGE index layout: i -> [i % 16, i // 16], replicated to 128 partitions."""
    n = idx.shape[0]
    assert n % 16 == 0
    w = idx.astype(np.int16).reshape(n // 16, 16).T
    return np.ascontiguousarray(np.tile(w, (8, 1)))


def _prep_graph(ei):
    """Dense adjacency-count blocks (incl. self loops) + degree counts.

    Returns (A_blocks, deg): A_blocks[t, p, sc, q] = #edges src=sc*128+p ->
    dst=t*128+q, laid out so A_blocks[t] is directly the stack of matmul
    lhsT tiles for dst-tile t. deg includes the self loop. Counts are small
    integers so fp8e4 (e4m3) stores them exactly.
    """
    s = np.concatenate([ei[0].astype(np.int64), np.arange(N, dtype=np.int64)])
    d = np.concatenate([ei[1].astype(np.int64), np.arange(N, dtype=np.int64)])
    A = np.zeros((N, N), np.float32)
    np.add.at(A, (d, s), 1.0)
    deg = np.bincount(d, minlength=N).astype(np.int32)
    Ab = np.ascontiguousarray(
        A.reshape(NT, P, NT, P).transpose(0, 3, 2, 1)
    ).astype(f8e4)
    return Ab, deg


def kernel(**inputs):
    import concourse.bacc as bacc
    import concourse.mybir as mybir
    import concourse.tile as tile
    from concourse.bass_utils import run_bass_kernel_spmd

    f32, bft, fp8, i16, i32 = (
        mybir.dt.float32,
        mybir.dt.bfloat16,
        mybir.dt.float8e4,
        mybir.dt.int16,
        mybir.dt.int32,
    )
    DR = mybir.MatmulPerfMode.DoubleRow

    t_full = np.asarray(inputs["token_embeddings"], dtype=np.float32)
    W = {
        "e": np.asarray(inputs["W_edges"], dtype=np.float32),
        "n": np.asarray(inputs["W_nodes"], dtype=np.float32),
    }
    bias = {
        "e": np.asarray(inputs["b_edges"], dtype=np.float32),
        "n": np.asarray(inputs["b_nodes"], dtype=np.float32),
    }
    gate = {
        "e": np.asarray(inputs["gate_a"], dtype=np.float32).reshape(1, 1),
        "n": np.asarray(inputs["gate_b"], dtype=np.float32).reshape(1, 1),
    }
    t2x = {
        "e": np.asarray(inputs["tokens2edges"], dtype=np.int64),
        "n": np.asarray(inputs["tokens2nodes"], dtype=np.int64),
    }
    x2t = {
        "e": np.asarray(inputs["edges2tokens"], dtype=np.int64),
        "n": np.asarray(inputs["nodes2tokens"], dtype=np.int64),
    }
    ei = {
        "e": np.asarray(inputs["edge_index_edges"], dtype=np.int64),
        "n": np.asarray(inputs["edge_index_nodes"], dtype=np.int64),
    }

    gcns = ("e", "n")

    graphs = {g: [_prep_graph(ei[g][b]) for b in range(B)] for g in gcns}

    # ---- per-core host data ----
    core_data = []
    for b in range(B):
        per_g = {}
        for g in gcns:
            Ab, deg = graphs[g][b]
            # out rows live at r = (tok % 128) * TOKC + tok // 128 so the
            # out-init writes are contiguous per partition; remap the scatter
            # targets accordingly.
            x2t_r = (x2t[g][b] % P) * TOKC + x2t[g][b] // P
            per_g[g] = dict(
                A=Ab,
                deg_pc=np.ascontiguousarray(deg.reshape(NT, P).T),
                t2x_w=_wrap_idx(t2x[g][b]),
                x2t_w=_wrap_idx(x2t_r),
            )
        t_f8 = t_full[b].astype(f8e4)
        for h in range(2):
            d = dict(t_f8=t_f8)
            for g in gcns:
                # x^T fp8 transpose-gather interleaves 16-bit units: element
                # d = 2*(slot*128 + p) + j lands at xT8[p, slot, tok, j]; pack
                # W rows to match.
                d[f"W_{g}"] = np.ascontiguousarray(
                    W[g][:, h * H : (h + 1) * H].reshape(3, P, 2, H).transpose(1, 0, 2, 3)
                ).astype(bf16)
                d[f"bias_{g}"] = np.ascontiguousarray(bias[g][None, h * H : (h + 1) * H])
                d[f"gate_{g}"] = gate[g]
                d.update({f"{k}_{g}": v for k, v in per_g[g].items()})
            d["t_half"] = np.ascontiguousarray(
                t_full[b].reshape(TOKC, P, D).transpose(1, 0, 2)[:, :, h * H : (h + 1) * H]
            ).astype(bf16)
            core_data.append(d)

    # ---- build the SPMD program ----
    nc = bacc.Bacc("TRN2", target_bir_lowering=False, debug=False, num_swdge_queues=1)

    t_f8_d = nc.declare_dram_parameter("t_f8", [S, D], fp8, isOutput=False)
    t_half_d = nc.declare_dram_parameter("t_half", [P, TOKC, H], bft, isOutput=False)
    ins_d = {}
    for g in gcns:
        ins_d[f"W_{g}"] = nc.declare_dram_parameter(f"W_{g}", [P, 3, 2, H], bft, isOutput=False)
        ins_d[f"bias_{g}"] = nc.declare_dram_parameter(f"bias_{g}", [1, H], f32, isOutput=False)
        ins_d[f"gate_{g}"] = nc.declare_dram_parameter(f"gate_{g}", [1, 1], f32, isOutput=False)
        ins_d[f"A_{g}"] = nc.declare_dram_parameter(
            f"A_{g}", [NT, P, NT, P], fp8, isOutput=False
        )
        ins_d[f"deg_pc_{g}"] = nc.declare_dram_parameter(
            f"deg_pc_{g}", [P, NT], i32, isOutput=False
        )
        ins_d[f"t2x_w_{g}"] = nc.declare_dram_parameter(
            f"t2x_w_{g}", [P, N // 16], i16, isOutput=False
        )
        ins_d[f"x2t_w_{g}"] = nc.declare_dram_parameter(
            f"x2t_w_{g}", [P, N // 16], i16, isOutput=False
        )
    out_d = nc.declare_dram_parameter("out", [S, H], bft, isOutput=True)

    out_rows = out_d.rearrange("(p c) h -> p c h", p=P)

    with tile.TileContext(nc) as tc:
        with (
            tc.tile_pool(name="cst", bufs=1) as cst,
            tc.tile_pool(name="idxp", bufs=1) as idxp,
            tc.tile_pool(name="xt", bufs=3) as xtp,
            tc.tile_pool(name="xtn", bufs=8) as xtnp,
            tc.tile_pool(name="yp", bufs=2) as yp,
            tc.tile_pool(name="ap", bufs=5) as apool,
            tc.tile_pool(name="ep", bufs=2) as ep,
            tc.tile_pool(name="tb", bufs=4) as tbp,
            tc.tile_pool(name="psxw", bufs=3, space="PSUM") as psxw,
            tc.tile_pool(name="psz", bufs=2, space="PSUM") as psz,
        ):
            # ---------- setup (index tiles first so gathers can start early) ----------
            Wsb, bias_ga, dinv, dinv_ga = {}, {}, {}, {}
            idx_t2x, idx_x2t = {}, {}
            for g in gcns:
                idx_t2x[g] = idxp.tile([P, N // 16], i16, name=f"it2x_{g}", tag=f"it2x_{g}")
                nc.sync.dma_start(out=idx_t2x[g][:], in_=ins_d[f"t2x_w_{g}"][:])
                idx_x2t[g] = idxp.tile([P, N // 16], i16, name=f"ix2t_{g}", tag=f"ix2t_{g}")
                nc.sync.dma_start(out=idx_x2t[g][:], in_=ins_d[f"x2t_w_{g}"][:])
            for g in gcns:
                Wsb[g] = cst.tile([P, 3, 2, H], bft, name=f"W_{g}", tag=f"W_{g}")
                nc.sync.dma_start(out=Wsb[g][:], in_=ins_d[f"W_{g}"][:])

                gcol = cst.tile([P, 1], f32, name=f"gcol_{g}", tag=f"gcol_{g}")
                nc.sync.dma_start(
                    out=gcol[:], in_=ins_d[f"gate_{g}"][:1, :].to_broadcast([P, 1])
                )
                tanh_g = cst.tile([P, 1], f32, name=f"tanh_{g}", tag=f"tanh_{g}")
                nc.scalar.activation(
                    out=tanh_g[:], in_=gcol[:], func=mybir.ActivationFunctionType.Tanh
                )

                brow = cst.tile([P, H], f32, name=f"brow_{g}", tag=f"brow_{g}")
                nc.sync.dma_start(
                    out=brow[:], in_=ins_d[f"bias_{g}"][:1, :].to_broadcast([P, H])
                )
                bias_ga[g] = cst.tile([P, H], f32, name=f"biasga_{g}", tag=f"biasga_{g}")
                nc.vector.tensor_scalar_mul(bias_ga[g][:], brow[:], tanh_g[:, :1])

                deg_i = cst.tile([P, NT], i32, name=f"degi_{g}", tag=f"degi_{g}")
                nc.sync.dma_start(out=deg_i[:], in_=ins_d[f"deg_pc_{g}"][:])
                deg_f = cst.tile([P, NT], f32, name=f"degf_{g}", tag=f"degf_{g}")
                nc.vector.tensor_copy(out=deg_f[:], in_=deg_i[:])
                rdeg = cst.tile([P, NT], f32, name=f"rdeg_{g}", tag=f"rdeg_{g}")
                nc.vector.reciprocal(rdeg[:], deg_f[:])
                dinv[g] = cst.tile([P, NT], f32, name=f"dinv_{g}", tag=f"dinv_{g}")
                nc.scalar.sqrt(dinv[g][:], rdeg[:])
                dinv_ga[g] = cst.tile([P, NT], f32, name=f"dinvga_{g}", tag=f"dinvga_{g}")
                nc.vector.tensor_scalar_mul(dinv_ga[g][:], dinv[g][:], tanh_g[:, :1])

            y_sb = {}
            for g in gcns:
                y_sb[g] = yp.tile([P, NT, H], fp8, name=f"ysb_{g}", tag="ysb")

            def out_init_wave(w):
                """out := t for one wave of OW token chunks (contiguous rows)."""
                tch = tbp.tile([P, OW, H], bft, name="tch", tag="tb")
                nc.sync.dma_start(
                    out=tch[:], in_=t_half_d[:, w * OW : (w + 1) * OW, :]
                )
                nc.sync.dma_start(
                    out=out_rows[:, w * OW : (w + 1) * OW, :], in_=tch[:]
                )

            def gather_wave(g, w, pool):
                xT = pool.tile([P, 6, XTG], fp8, name="xT")
                nc.gpsimd.dma_gather(
                    out_ap=xT[:],
                    in_ap=t_f8_d[:],
                    idxs_ap=idx_t2x[g][:, w * (XTG // 16) : (w + 1) * (XTG // 16)],
                    num_idxs=XTG,
                    num_idxs_reg=XTG,
                    elem_size=D,
                    transpose=True,
                    queue_num=0,
                )
                return xT

            def xw_chunks(g, w, xT):
                # fp8 transpose-gather lands element d = 2*(s*128+p)+j at
                # (p, s, tok, j); view the [P, 6, XTG] tile accordingly
                xv = xT[:].rearrange("p a b -> p (a b)").rearrange(
                    "p (s x two) -> p s x two", s=3, two=2
                )
                for c in range(XTG // P):
                    cc = w * (XTG // P) + c
                    ps = psxw.tile([P, H], f32)
                    for k in range(6):
                        s, j = k // 2, k % 2
                        nc.tensor.matmul(
                            out=ps[:],
                            lhsT=xv[:, s, c * P : (c + 1) * P, j],
                            rhs=Wsb[g][:, s, j, :],
                            start=(k == 0),
                            stop=(k == 5),
                        )
                    nc.scalar.activation(
                        out=y_sb[g][:, cc, :],
                        in_=ps[:],
                        func=mybir.ActivationFunctionType.Copy,
                        scale=dinv[g][:, cc : cc + 1],
                    )

            # ---------- aggregation for one dst tile (fp8 DoubleRow) ----------
            def agg_tile(g, t_i, e_grp, j):
                At = apool.tile([P, NT, P], fp8, name="At", tag="At")
                nc.sync.dma_start(out=At[:], in_=ins_d[f"A_{g}"][t_i])
                zt = psz.tile([P, H], f32, name="zt", tag="zt")
                for i in range(NT // 2):
                    nc.tensor.matmul(
                        out=zt[:],
                        lhsT=At[:, 2 * i : 2 * i + 2, :],
                        rhs=y_sb[g][:, 2 * i : 2 * i + 2, :],
                        start=(i == 0),
                        stop=(i == NT // 2 - 1),
                        perf_mode=DR,
                    )
                nc.vector.scalar_tensor_tensor(
                    out=e_grp[:, j, :],
                    in0=zt[:],
                    scalar=dinv_ga[g][:, t_i : t_i + 1],
                    in1=bias_ga[g][:],
                    op0=mybir.AluOpType.mult,
                    op1=mybir.AluOpType.add,
                )

            def agg_grp(g, t0, ntiles):
                """Aggregate dst tiles [t0, t0+ntiles) and scatter-add into out."""
                e_grp = ep.tile([P, ntiles, H], bft, name="e_grp", tag=f"egrp{ntiles}")
                for j in range(ntiles):
                    agg_tile(g, t0 + j, e_grp, j)
                nc.gpsimd.dma_scatter_add(
                    out_ap=out_d[:, :],
                    in_ap=e_grp[:],
                    idxs_ap=idx_x2t[g][
                        :, t0 * (P // 16) : (t0 + ntiles) * (P // 16)
                    ],
                    num_idxs=ntiles * P,
                    num_idxs_reg=ntiles * P,
                    elem_size=H,
                    queue_num=0,
                )

            # phase 0: out := t up front — every scatter-add WAW-depends on the
            # last of these writes, so they must complete early
            for w in range(TOKC // OW):
                out_init_wave(w)
            # phase A: xw for GCN e
            for w in range(N // XTG):
                xT = gather_wave("e", w, xtp)
                xw_chunks("e", w, xT)
            # phase B: prefetch ALL GCN-n gathers before any scatter is enqueued
            # on the gpsimd engine (avoids head-of-line blocking behind scatters)
            xTn = [gather_wave("n", w, xtnp) for w in range(N // XTG)]
            # phase C: interleave xw-n with agg-e groups
            for w in range(N // XTG):
                xw_chunks("n", w, xTn[w])
                agg_grp("e", w * SCG, SCG)
            # phase D: agg-n; final group is a single tile so the last
            # scatter-add chain link after compute ends is short
            for grp in range(NT // SCG - 1):
                agg_grp("n", grp * SCG, SCG)
            agg_grp("n", NT - SCG, 3)
            agg_grp("n", NT - 1, 1)

    nc.compile()

    in_maps = [{k: v for k, v in cd.items()} for cd in core_data]
    global LAST_RESULT
    kw = {}
    if TRACE:
        kw = dict(trace=True, trace_cores=TRACE_CORES, stitch_traces=False)
    res = run_bass_kernel_spmd(nc, in_maps, list(range(8)), **kw)
    LAST_RESULT = res

    out = np.empty((B, S, D), np.float32)
    for b in range(B):
        for h in range(2):
            o = np.asarray(res.results[2 * b + h]["out"], dtype=np.float32)
            out[b, :, h * H : (h + 1) * H] = (
                o.reshape(P, TOKC, H).transpose(1, 0, 2).reshape(S, H)
            )
    return out
